# Initial kernel scaffold
#
"""Trainium2 Bass kernel for DoubleGraphConvNet (gnn_message_passing).

Strategy (8 NeuronCores, SPMD single program):
- Nodes of each branch are dst-sharded across 8 cores using a padded
  numbering (core c owns padded rows [c*S, (c+1)*S)); host builds a
  permutation balancing per-tile edge counts.
- segment_sum: per 128-edge chunk, indirect-DMA gather of source rows
  (bf16) + DVE is_equal one-hot + PSUM-accumulated matmul, producing the
  aggregate feature-major [c, 128] directly.
- Layers 1-2 aggregate-first, layer 3 transform-first (y3 = x3 @ W3r.T
  computed per-shard, then AllGather + gather of y3 rows).
- Cross-core exchange: AllGather of bf16 activations after L1 and of y3;
  final AllReduce of pooled sums; the small MLP is replicated on-device.
"""
import os
import sys
import math

sys.path.insert(0, "/opt/trn_rl_repo")

import numpy as np
import ml_dtypes

import concourse.bass as bass
import concourse.mybir as mybir
import concourse.tile as tile
from concourse import bacc
from concourse.bass_utils import run_bass_kernel_spmd

P = 128
NCORES = 8
B = 8
BF16 = mybir.dt.bfloat16
F32 = mybir.dt.float32
I32 = mybir.dt.int32
AF = mybir.ActivationFunctionType
ALU = mybir.AluOpType

CFG_FULL = dict(N_G=50000, E_G=800000, N_S=10000, E_S=160000, NF=64)


# ---------------------------------------------------------------------------
# host-side preprocessing
# ---------------------------------------------------------------------------

def _assign_nodes(dst, n_nodes):
    """Balanced node -> (core, slot, pos) assignment.

    Returns perm (node -> padded id), slots (per core), nch[slot] (shared
    chunk counts), and per-core edge arrays will be built from this.
    """
    import heapq
    deg = np.bincount(dst, minlength=n_nodes)
    slots = math.ceil(n_nodes / (NCORES * P))
    nbins = NCORES * slots
    order = np.argsort(-deg, kind="stable")
    # greedy least-loaded bin with capacity 128
    heap = [(0, 0, b) for b in range(nbins)]
    heapq.heapify(heap)
    bin_of = np.empty(n_nodes, np.int64)
    bin_fill = np.zeros(nbins, np.int64)
    pos_of = np.empty(n_nodes, np.int64)
    for n in order:
        while True:
            load, cnt, b = heapq.heappop(heap)
            if bin_fill[b] < P:
                break
        bin_of[n] = b
        pos_of[n] = bin_fill[b]
        bin_fill[b] += 1
        if bin_fill[b] < P:
            heapq.heappush(heap, (load + int(deg[n]), cnt + 1, b))
    # bins -> (core, slot): order bins per core by edge load desc so heavy
    # slots align across cores (minimizes sum of per-slot maxima)
    bin_load = np.zeros(nbins, np.int64)
    np.add.at(bin_load, bin_of[dst], 1)
    core_of_bin = np.arange(nbins) % NCORES  # spread bins round-robin
    slot_of_bin = np.empty(nbins, np.int64)
    for c in range(NCORES):
        bins_c = np.where(core_of_bin == c)[0]
        order_c = bins_c[np.argsort(-bin_load[bins_c], kind="stable")]
        slot_of_bin[order_c] = np.arange(slots)
    core_of = core_of_bin[bin_of]
    slot_of = slot_of_bin[bin_of]
    perm = core_of * (slots * P) + slot_of * P + pos_of
    return perm, core_of, slot_of, pos_of, slots


def _build_edges(src, dst, perm, core_of, slot_of, pos_of, slots):
    """Per-core edge arrays in [P, total_chunks] layout + shared nch."""
    ecore = core_of[dst]
    eslot = slot_of[dst]
    # counts per (core, slot)
    cnt = np.zeros((NCORES, slots), np.int64)
    np.add.at(cnt, (ecore, eslot), 1)
    nch = np.maximum(1, np.ceil(cnt.max(axis=0) / P).astype(np.int64))
    tc = int(nch.sum())
    starts = np.concatenate([[0], np.cumsum(nch)])[:-1]  # chunk start per slot
    src_arr = np.zeros((NCORES, P, tc), np.int32)
    dstl_arr = np.full((NCORES, P, tc), 255.0, np.float32)
    order = np.lexsort((eslot, ecore))
    es, ed, ec, esl = src[order], dst[order], ecore[order], eslot[order]
    psrc = perm[es].astype(np.int32)
    pdst = pos_of[ed].astype(np.float32)
    # group boundaries
    key = ec * slots + esl
    bounds = np.searchsorted(key, np.arange(NCORES * slots + 1))
    for c in range(NCORES):
        for s in range(slots):
            k = c * slots + s
            a, b = bounds[k], bounds[k + 1]
            n = b - a
            if n == 0:
                continue
            c0 = int(starts[s])
            seg_src = psrc[a:b]
            seg_dst = pdst[a:b]
            # fill column-major into [P, nch[s]] region
            ncol = int(nch[s])
            buf_s = np.zeros(P * ncol, np.int32)
            buf_d = np.full(P * ncol, 255.0, np.float32)
            buf_s[:n] = seg_src
            buf_d[:n] = seg_dst
            src_arr[c, :, c0:c0 + ncol] = buf_s.reshape(ncol, P).T
            dstl_arr[c, :, c0:c0 + ncol] = buf_d.reshape(ncol, P).T
    return src_arr, dstl_arr, nch.astype(int), tc


def _prep_branch(x, edge_index, batch, n_nodes):
    src = np.asarray(edge_index[0], np.int64)
    dst = np.asarray(edge_index[1], np.int64)
    perm, core_of, slot_of, pos_of, slots = _assign_nodes(dst, n_nodes)
    src_arr, dstl_arr, nch, tc = _build_edges(
        src, dst, perm, core_of, slot_of, pos_of, slots)
    npad = NCORES * slots * P
    S = slots * P
    nf = x.shape[1]
    x_full = np.zeros((npad, nf), np.float32)
    x_full[perm] = np.asarray(x, np.float32)
    # per-core own xT [nf, S]
    xT = np.stack([x_full[c * S:(c + 1) * S].T.copy() for c in range(NCORES)])
    # pooling onehot [P, slots*B] per core & counts
    ohB = np.zeros((NCORES, P, slots * B), np.float32)
    bvec = np.asarray(batch, np.int64)
    for n in range(n_nodes):
        pid = perm[n]
        c, r = divmod(pid, S)
        s, p = divmod(r, P)
        ohB[c, p, s * B + int(bvec[n])] = 1.0
    cntb = np.bincount(bvec, minlength=B).astype(np.float32)
    recip = 1.0 / np.maximum(cntb, 1.0)
    return dict(perm=perm, slots=slots, S=S, npad=npad, nch=nch, tc=tc,
                src_arr=src_arr, dstl_arr=dstl_arr, x_full=x_full, xT=xT,
                ohB=ohB, recip=recip)


def _pack_wt(w):
    """[o, c] weight -> transposed [c, o] f32 (host)."""
    return np.ascontiguousarray(np.asarray(w, np.float32).T)


def _pack_bias(bvec, nchunks):
    out = np.zeros((P, nchunks), np.float32)
    b = np.asarray(bvec, np.float32)
    for j in range(nchunks):
        seg = b[j * P:(j + 1) * P]
        out[:len(seg), j] = seg
    return out


def host_prep(inputs, cfg):
    g = _prep_branch(inputs["graph_x"], inputs["graph_edge_index"],
                     inputs["graph_batch"], cfg["N_G"])
    s = _prep_branch(inputs["subgraph_x"], inputs["subgraph_edge_index"],
                     inputs["subgraph_batch"], cfg["N_S"])
    NF = cfg["NF"]
    meta = dict(g=g, s=s, NF=NF)

    bf = ml_dtypes.bfloat16
    common = {}
    common["iota"] = np.broadcast_to(
        np.arange(P, dtype=np.float32), (P, P)).copy()
    common["ident"] = np.eye(P, dtype=np.float32)
    common["identb"] = np.eye(P, dtype=bf)

    # conv weights: for pre in (g, s): W{l}r/W{l}n [o, c] -> WT [c, o] bf16
    dims = [(2 * NF, NF), (4 * NF, 2 * NF), (3 * NF, 4 * NF)]
    meta["dims"] = dims
    for pre in ("g", "s"):
        for li, (o, c) in enumerate(dims, start=1):
            common[f"{pre}W{li}rT"] = _pack_wt(inputs[f"{pre}W{li}r"]).astype(bf)
            common[f"{pre}W{li}nT"] = _pack_wt(inputs[f"{pre}W{li}n"]).astype(bf)
            common[f"{pre}B{li}"] = _pack_bias(inputs[f"{pre}B{li}"],
                                               math.ceil(o / P))
    # MLP: input order [g_pool(3NF) | s_pool(3NF) | point(NF)] -> zT slots:
    # slot rows: [g0:128, g1:64+pad, s0:128, s1:64+pad, p:64+pad] = 640
    O3 = 3 * NF  # 192
    zmap = np.full(5 * P, -1, np.int64)
    zmap[0:P] = np.arange(0, P)
    zmap[P:P + (O3 - P)] = np.arange(P, O3)
    zmap[2 * P:3 * P] = O3 + np.arange(0, P)
    zmap[3 * P:3 * P + (O3 - P)] = O3 + np.arange(P, O3)
    zmap[4 * P:4 * P + NF] = 2 * O3 + np.arange(NF)
    l1W = np.asarray(inputs["l1W"], np.float32)  # [600, 448]
    l1WT = np.zeros((5 * P, 600), np.float32)
    valid = zmap >= 0
    l1WT[valid] = l1W[:, zmap[valid]].T
    H1, H2 = 600, 256
    K1 = 5  # k chunks of 128 over 640
    M1 = math.ceil(H1 / P)  # 5 chunks over 600
    l2W = np.asarray(inputs["l2W"], np.float32)  # [256, 600]
    l2WT = np.zeros((M1 * P, H2), np.float32)
    l2WT[:H1] = l2W.T
    l3W = np.asarray(inputs["l3W"], np.float32)  # [64, 256]
    l3WT = np.ascontiguousarray(l3W.T)  # [256, 64]

    def pack_k(wt, kchunks, width):
        out = np.zeros((P, kchunks * width), np.float32)
        for k in range(kchunks):
            seg = wt[k * P:(k + 1) * P]
            out[:seg.shape[0], k * width:k * width + width] = seg
        return out

    common["l1WT"] = pack_k(l1WT, K1, 600)
    common["l2WT"] = pack_k(l2WT, M1, H2)
    common["l3WT"] = pack_k(l3WT, 2, NF)
    common["l1b"] = _pack_bias(inputs["l1b"], M1)
    common["l2b"] = _pack_bias(inputs["l2b"], 2)
    common["l3b"] = _pack_bias(inputs["l3b"], 1)
    common["pointT"] = np.ascontiguousarray(
        np.asarray(inputs["point"], np.float32).T)  # [NF, B]
    common["xg1_full"] = g["x_full"].astype(bf)
    common["xs1_full"] = s["x_full"].astype(bf)

    in_maps = []
    for c in range(NCORES):
        m = dict(common)
        m["g_src"] = g["src_arr"][c]
        m["g_dstl"] = g["dstl_arr"][c]
        m["s_src"] = s["src_arr"][c]
        m["s_dstl"] = s["dstl_arr"][c]
        m["xg1T"] = g["xT"][c].astype(bf)
        m["xs1T"] = s["xT"][c].astype(bf)
        m["g_ohB"] = g["ohB"][c]
        m["s_ohB"] = s["ohB"][c]
        rg = np.broadcast_to(g["recip"], (P, B)).astype(np.float32).copy()
        rs = np.broadcast_to(s["recip"], (P, B)).astype(np.float32).copy()
        m["g_recip"] = rg
        m["s_recip"] = rs
        in_maps.append(m)
    return meta, in_maps


# ---------------------------------------------------------------------------
# device program
# ---------------------------------------------------------------------------

def _ap3(t_ap, mid_count):
    """[P, X] AP -> [P, mid_count, X] with stride-0 middle dim."""
    return bass.AP(t_ap.tensor, t_ap.offset,
                   [list(t_ap.ap[0]), [0, mid_count], list(t_ap.ap[1])])


def build_program(meta):
    NF = meta["NF"]
    dims = meta["dims"]
    g, s = meta["g"], meta["s"]
    O3 = 3 * NF

    nc = bacc.Bacc(None, target_bir_lowering=False, debug=False)
    dt_in = {}

    def din(name, shape, dtype):
        t = nc.dram_tensor(name, list(shape), dtype, kind="ExternalInput")
        dt_in[name] = t
        return t

    # inputs
    xg1_full = din("xg1_full", g["x_full"].shape, BF16)
    xs1_full = din("xs1_full", s["x_full"].shape, BF16)
    xg1T = din("xg1T", [NF, g["S"]], BF16)
    xs1T = din("xs1T", [NF, s["S"]], BF16)
    g_src = din("g_src", [P, g["tc"]], I32)
    g_dstl = din("g_dstl", [P, g["tc"]], F32)
    s_src = din("s_src", [P, s["tc"]], I32)
    s_dstl = din("s_dstl", [P, s["tc"]], F32)
    iota_in = din("iota", [P, P], F32)
    ident_in = din("ident", [P, P], F32)
    identb_in = din("identb", [P, P], BF16)
    wts = {}
    for pre in ("g", "s"):
        for li, (o, c) in enumerate(dims, start=1):
            wts[f"{pre}W{li}rT"] = din(f"{pre}W{li}rT", [c, o], BF16)
            wts[f"{pre}W{li}nT"] = din(f"{pre}W{li}nT", [c, o], BF16)
            wts[f"{pre}B{li}"] = din(f"{pre}B{li}", [P, math.ceil(o / P)], F32)
    l1WT = din("l1WT", [P, 5 * 600], F32)
    l2WT = din("l2WT", [P, 5 * 256], F32)
    l3WT = din("l3WT", [P, 2 * NF], F32)
    l1b = din("l1b", [P, 5], F32)
    l2b = din("l2b", [P, 2], F32)
    l3b = din("l3b", [P, 1], F32)
    pointT = din("pointT", [NF, B], F32)
    g_ohB = din("g_ohB", [P, g["slots"] * B], F32)
    s_ohB = din("s_ohB", [P, s["slots"] * B], F32)
    g_recip = din("g_recip", [P, B], F32)
    s_recip = din("s_recip", [P, B], F32)

    out_ext = nc.dram_tensor("out", [B, NF], F32, kind="ExternalOutput")

    with tile.TileContext(nc) as tc:
        with tc.tile_pool(name="const", bufs=1) as cp, \
             tc.tile_pool(name="gat", bufs=3) as gat_p, \
             tc.tile_pool(name="oh", bufs=3) as oh_p, \
             tc.tile_pool(name="evac", bufs=3) as ev_p, \
             tc.tile_pool(name="elu", bufs=3) as elu_p, \
             tc.tile_pool(name="stage", bufs=3) as st_p, \
             tc.tile_pool(name="psA", bufs=3, space="PSUM") as psA, \
             tc.tile_pool(name="psB", bufs=3, space="PSUM") as psB, \
             tc.tile_pool(name="psT", bufs=2, space="PSUM") as psT, \
             tc.tile_pool(name="dram", bufs=1, space="DRAM") as dram:

            # ------- constants in SBUF -------
            def load_const(name, src_t, shape, dtype):
                t = cp.tile(list(shape), dtype, tag=name)
                nc.sync.dma_start(out=t[:], in_=src_t[:])
                return t

            iota_t = load_const("iota", iota_in, [P, P], F32)
            ident_t = load_const("ident", ident_in, [P, P], F32)
            identb_t = load_const("identb", identb_in, [P, P], BF16)
            w_t = {}
            for pre in ("g", "s"):
                for li, (o, c) in enumerate(dims, start=1):
                    for rn in ("r", "n"):
                        nm = f"{pre}W{li}{rn}T"
                        kch = math.ceil(c / P)
                        t = cp.tile([P, kch * o], BF16, tag=nm)
                        src_w = wts[nm]
                        ap = src_w[:].rearrange("(k p) o -> p (k o)", p=min(P, c))
                        if c < P:
                            nc.sync.dma_start(out=t[:c, :o], in_=src_w[:])
                        else:
                            nc.sync.dma_start(out=t[:], in_=ap)
                        w_t[nm] = t
                    nm = f"{pre}B{li}"
                    w_t[nm] = load_const(nm, wts[nm],
                                         [P, math.ceil(o / P)], F32)
            l1w_t = load_const("l1WT", l1WT, [P, 5 * 600], F32)
            l2w_t = load_const("l2WT", l2WT, [P, 5 * 256], F32)
            l3w_t = load_const("l3WT", l3WT, [P, 2 * NF], F32)
            l1b_t = load_const("l1b", l1b, [P, 5], F32)
            l2b_t = load_const("l2b", l2b, [P, 2], F32)
            l3b_t = load_const("l3b", l3b, [P, 1], F32)
            pointT_t = load_const("pointT", pointT, [NF, B], F32)
            gohB_t = load_const("g_ohB", g_ohB, [P, g["slots"] * B], F32)
            sohB_t = load_const("s_ohB", s_ohB, [P, s["slots"] * B], F32)
            grec_t = load_const("g_recip", g_recip, [P, B], F32)
            srec_t = load_const("s_recip", s_recip, [P, B], F32)

            gsrc_t = load_const("g_src", g_src, [P, g["tc"]], I32)
            gdstl_t = load_const("g_dstl", g_dstl, [P, g["tc"]], F32)
            ssrc_t = load_const("s_src", s_src, [P, s["tc"]], I32)
            sdstl_t = load_const("s_dstl", s_dstl, [P, s["tc"]], F32)

            # persistent xT stores (2 f-chunks max)
            xT_store = {}
            for pre, br in (("g", g), ("s", s)):
                for ab in "AB":
                    xT_store[pre + ab] = cp.tile(
                        [P, 2 * br["slots"] * P], BF16, tag=f"xT{pre}{ab}")
            # L1 xT: copy from input [NF, S] into store A chunk 0
            nc.scalar.copy(out=xT_store["gA"][:NF, :g["S"]], in_=xg1T[:])
            nc.scalar.copy(out=xT_store["sA"][:NF, :s["S"]], in_=xs1T[:])

            # DRAM intermediates
            dims_l2 = dims[1]  # (4NF, 2NF)
            xg2_shard = dram.tile([g["S"], 2 * NF], BF16, tag="xg2_shard")
            xg2_full = dram.tile([g["npad"], 2 * NF], BF16, tag="xg2_full")
            xs2_shard = dram.tile([s["S"], 2 * NF], BF16, tag="xs2_shard")
            xs2_full = dram.tile([s["npad"], 2 * NF], BF16, tag="xs2_full")
            yg3_shard = dram.tile([g["S"], O3], BF16, tag="yg3_shard")
            yg3_full = dram.tile([g["npad"], O3], BF16, tag="yg3_full")
            ys3_shard = dram.tile([s["S"], O3], BF16, tag="ys3_shard")
            ys3_full = dram.tile([s["npad"], O3], BF16, tag="ys3_full")
            ar_in = dram.tile([P, 4 * B], F32, tag="ar_in")
            ar_out = dram.tile([P, 4 * B], F32, tag="ar_out")

            pool_ps = {}  # pooled psum tiles, accumulated across slots

            # ---------------- layer emitters ----------------
            def emit_layer(pre, br, li, x_full_t, src_t, dstl_t,
                           xin_store, xout_store, shard_t, pool_phase):
                """One GraphConv layer for one branch.

                li: 1..3. For li in (1,2): aggregate-first; li==3:
                gathers y3 (already W-transformed), adds Wn term, ELU,
                then pooling.
                """
                o, c = dims[li - 1]
                cg = c if li < 3 else O3     # gathered row width
                och = math.ceil(o / P)
                kch = math.ceil(c / P)
                nch = br["nch"]
                starts = np.concatenate([[0], np.cumsum(nch)]).astype(int)
                WrT = w_t[f"{pre}W{li}rT"]
                WnT = w_t[f"{pre}W{li}nT"]
                bias = w_t[f"{pre}B{li}"]
                gch = math.ceil(cg / P)      # f-chunks of gathered rows

                for slot in range(br["slots"]):
                    n_j = int(nch[slot])
                    c0 = int(starts[slot])
                    # gather chunk rows
                    g_t = gat_p.tile([P, 17 * 256], BF16, tag="gat")
                    for j in range(n_j):
                        nc.gpsimd.indirect_dma_start(
                            out=g_t[:, j * cg:(j + 1) * cg],
                            out_offset=None,
                            in_=x_full_t[:],
                            in_offset=bass.IndirectOffsetOnAxis(
                                ap=src_t[:, c0 + j:c0 + j + 1], axis=0))
                    # onehot for all chunks of this slot
                    oh_t = oh_p.tile([P, 17 * P], BF16, tag="oh")
                    d_ap = dstl_t[:, c0:c0 + n_j].to_broadcast([P, n_j, P])
                    i_ap = _ap3(iota_t[:], n_j)
                    nc.vector.tensor_tensor(
                        out=oh_t[:, :n_j * P].rearrange(
                            "p (k q) -> p k q", k=n_j),
                        in0=d_ap, in1=i_ap, op=ALU.is_equal)

                    if li < 3:
                        # aggregate into aggT [c, P] psum (input space)
                        aggps = []
                        for kc in range(kch):
                            m0, m1 = kc * P, min(c, (kc + 1) * P)
                            ap_t = psA.tile([m1 - m0, P], F32, space="PSUM",
                                            tag=f"agg{kc}")
                            aggps.append((ap_t, m0, m1))
                        for j in range(n_j):
                            for (ap_t, m0, m1) in aggps:
                                nc.tensor.matmul(
                                    out=ap_t[:],
                                    lhsT=g_t[:, j * cg + m0:j * cg + m1],
                                    rhs=oh_t[:, j * P:(j + 1) * P],
                                    start=(j == 0), stop=(j == n_j - 1))
                        aggsb = ev_p.tile([P, kch * P], BF16, tag="aggsb")
                        for (ap_t, m0, m1) in aggps:
                            nc.scalar.copy(
                                out=aggsb[:m1 - m0, (m0 // P) * P:(m0 // P) * P + P],
                                in_=ap_t[:])
                        # out psum [o-chunk, P] = WrT.T @ aggT + WnT.T @ xT
                        outps = []
                        for oc in range(och):
                            o0, o1 = oc * P, min(o, (oc + 1) * P)
                            op_t = psB.tile([o1 - o0, P], F32, space="PSUM",
                                            tag=f"out{oc}")
                            outps.append((op_t, o0, o1))
                        for (op_t, o0, o1) in outps:
                            first = True
                            for kc in range(kch):
                                k0, k1 = kc * P, min(c, (kc + 1) * P)
                                nc.tensor.matmul(
                                    out=op_t[:],
                                    lhsT=WrT[:k1 - k0, kc * o + o0:kc * o + o1],
                                    rhs=aggsb[:k1 - k0, kc * P:kc * P + P],
                                    start=first, stop=False)
                                first = False
                                nc.tensor.matmul(
                                    out=op_t[:],
                                    lhsT=WnT[:k1 - k0, kc * o + o0:kc * o + o1],
                                    rhs=xin_store[:k1 - k0,
                                                  kc * br["slots"] * P
                                                  + slot * P:
                                                  kc * br["slots"] * P
                                                  + slot * P + P],
                                    start=False,
                                    stop=(kc == kch - 1))
                    else:
                        # li == 3: gathered rows already in out space (o = O3)
                        outps = []
                        for oc in range(och):
                            o0, o1 = oc * P, min(o, (oc + 1) * P)
                            op_t = psB.tile([o1 - o0, P], F32, space="PSUM",
                                            tag=f"out{oc}")
                            outps.append((op_t, o0, o1))
                        for j in range(n_j):
                            for (op_t, o0, o1) in outps:
                                nc.tensor.matmul(
                                    out=op_t[:],
                                    lhsT=g_t[:, j * cg + o0:j * cg + o1],
                                    rhs=oh_t[:, j * P:(j + 1) * P],
                                    start=(j == 0), stop=False)
                        for (op_t, o0, o1) in outps:
                            for kc in range(kch):
                                k0, k1 = kc * P, min(c, (kc + 1) * P)
                                nc.tensor.matmul(
                                    out=op_t[:],
                                    lhsT=WnT[:k1 - k0, kc * o + o0:kc * o + o1],
                                    rhs=xin_store[:k1 - k0,
                                                  kc * br["slots"] * P
                                                  + slot * P:
                                                  kc * br["slots"] * P
                                                  + slot * P + P],
                                    start=False,
                                    stop=(kc == kch - 1))

                    # ELU per o-chunk: out = relu(p+b) + exp(min(p+b,0)) - 1
                    for ci, (op_t, o0, o1) in enumerate(outps):
                        m = o1 - o0
                        b_ap = bias[:m, ci:ci + 1]
                        tmin = elu_p.tile([P, P], F32, tag="tmin")
                        nc.vector.tensor_scalar(
                            out=tmin[:m, :], in0=op_t[:], scalar1=b_ap,
                            scalar2=0.0, op0=ALU.add, op1=ALU.min)
                        texp = elu_p.tile([P, P], F32, tag="texp")
                        nc.scalar.activation(texp[:m, :], tmin[:m, :], AF.Exp)
                        trelu = elu_p.tile([P, P], F32, tag="trelu")
                        nc.scalar.activation(trelu[:m, :], op_t[:], AF.Relu,
                                             bias=b_ap)
                        tsum = elu_p.tile([P, P], F32, tag="tsum")
                        nc.vector.tensor_tensor(
                            out=tsum[:m, :], in0=trelu[:m, :],
                            in1=texp[:m, :], op=ALU.add)
                        if li < 3:
                            # bf16 xT for next layer
                            nc.vector.tensor_scalar(
                                out=xout_store[:m,
                                               ci * br["slots"] * P + slot * P:
                                               ci * br["slots"] * P + slot * P + P],
                                in0=tsum[:m, :], scalar1=-1.0, scalar2=None,
                                op0=ALU.add)
                            if shard_t is not None:
                                # node-major write: transpose + cast
                                tps = psT.tile([P, P], F32, space="PSUM",
                                               tag="tps")
                                telu = elu_p.tile([P, P], F32, tag="telu")
                                nc.vector.tensor_scalar(
                                    out=telu[:m, :], in0=tsum[:m, :],
                                    scalar1=-1.0, scalar2=None, op0=ALU.add)
                                nc.tensor.transpose(
                                    out=tps[:, :m], in_=telu[:m, :],
                                    identity=ident_t[:])
                                stg = st_p.tile([P, P], BF16, tag="stg")
                                nc.scalar.copy(out=stg[:, :m], in_=tps[:, :m])
                                nc.sync.dma_start(
                                    out=shard_t[slot * P:(slot + 1) * P,
                                                o0:o1],
                                    in_=stg[:, :m])
                        else:
                            telu = elu_p.tile([P, P], F32, tag="telu")
                            nc.vector.tensor_scalar(
                                out=telu[:m, :], in0=tsum[:m, :],
                                scalar1=-1.0, scalar2=None, op0=ALU.add)
                            # pooling: transpose to node-major, matmul ohB
                            tps = psT.tile([P, P], F32, space="PSUM",
                                           tag="tps")
                            nc.tensor.transpose(
                                out=tps[:, :m], in_=telu[:m, :],
                                identity=ident_t[:])
                            x4nm = st_p.tile([P, P], F32, tag="x4nm")
                            nc.scalar.copy(out=x4nm[:, :m], in_=tps[:, :m])
                            key = (pre, ci)
                            if key not in pool_ps:
                                pool_ps[key] = psA.tile(
                                    [m, B], F32, space="PSUM",
                                    tag=f"pool{pre}{ci}")
                            ohB_t = gohB_t if pre == "g" else sohB_t
                            nc.tensor.matmul(
                                out=pool_ps[key][:],
                                lhsT=x4nm[:, :m],
                                rhs=ohB_t[:, slot * B:(slot + 1) * B],
                                start=(slot == 0), stop=(slot == br["slots"] - 1))

            def emit_y3(pre, br, xin_store, shard_t):
                """y3 = x3 @ W3r.T per slot, feature-major, write shard."""
                o, c = dims[2]
                WrT = w_t[f"{pre}W3rT"]
                kch = math.ceil(c / P)
                och = math.ceil(o / P)
                for slot in range(br["slots"]):
                    for oc in range(och):
                        o0, o1 = oc * P, min(o, (oc + 1) * P)
                        yps = psB.tile([o1 - o0, P], F32, space="PSUM",
                                       tag=f"y{oc}")
                        for kc in range(kch):
                            k0, k1 = kc * P, min(c, (kc + 1) * P)
                            nc.tensor.matmul(
                                out=yps[:],
                                lhsT=WrT[:k1 - k0, kc * o + o0:kc * o + o1],
                                rhs=xin_store[:k1 - k0,
                                              kc * br["slots"] * P + slot * P:
                                              kc * br["slots"] * P + slot * P + P],
                                start=(kc == 0), stop=(kc == kch - 1))
                        ysb = elu_p.tile([P, P], F32, tag="ysb")
                        nc.scalar.copy(out=ysb[:o1 - o0, :], in_=yps[:])
                        tps = psT.tile([P, P], F32, space="PSUM", tag="tps")
                        nc.tensor.transpose(out=tps[:, :o1 - o0],
                                            in_=ysb[:o1 - o0, :],
                                            identity=ident_t[:])
                        stg = st_p.tile([P, P], BF16, tag="stg")
                        nc.scalar.copy(out=stg[:, :o1 - o0], in_=tps[:, :o1 - o0])
                        nc.sync.dma_start(
                            out=shard_t[slot * P:(slot + 1) * P, o0:o1],
                            in_=stg[:, :o1 - o0])

            def ag(shard_t, full_t):
                nc.gpsimd.collective_compute(
                    "AllGather", ALU.bypass,
                    replica_groups=[list(range(NCORES))],
                    ins=[shard_t.opt()], outs=[full_t.opt()])

            # ---------------- schedule ----------------
            # L1 graph -> AG ; L1 sub -> AG ; L2 graph; y3 g -> AG; L2 sub;
            # y3 s -> AG; L3 graph (pool); L3 sub (pool); AR; MLP
            emit_layer("g", g, 1, xg1_full, gsrc_t, gdstl_t,
                       xT_store["gA"], xT_store["gB"], xg2_shard, False)
            ag(xg2_shard, xg2_full)
            emit_layer("s", s, 1, xs1_full, ssrc_t, sdstl_t,
                       xT_store["sA"], xT_store["sB"], xs2_shard, False)
            ag(xs2_shard, xs2_full)
            emit_layer("g", g, 2, xg2_full, gsrc_t, gdstl_t,
                       xT_store["gB"], xT_store["gA"], None, False)
            emit_y3("g", g, xT_store["gA"], yg3_shard)
            ag(yg3_shard, yg3_full)
            emit_layer("s", s, 2, xs2_full, ssrc_t, sdstl_t,
                       xT_store["sB"], xT_store["sA"], None, False)
            emit_y3("s", s, xT_store["sA"], ys3_shard)
            ag(ys3_shard, ys3_full)
            emit_layer("g", g, 3, yg3_full, gsrc_t, gdstl_t,
                       xT_store["gA"], None, None, True)
            emit_layer("s", s, 3, ys3_full, ssrc_t, sdstl_t,
                       xT_store["sA"], None, None, True)

            # pack pooled sums -> AR -> recip -> zT
            arsb = cp.tile([P, 4 * B], F32, tag="arsb")
            nc.vector.memset(arsb[:], 0.0)
            blocks = [("g", 0, P), ("g", 1, O3 - P), ("s", 0, P),
                      ("s", 1, O3 - P)]
            for bi, (pre, ci, m) in enumerate(blocks):
                nc.vector.tensor_copy(out=arsb[:m, bi * B:(bi + 1) * B],
                                      in_=pool_ps[(pre, ci)][:])
            nc.sync.dma_start(out=ar_in[:], in_=arsb[:])
            nc.gpsimd.collective_compute(
                "AllReduce", ALU.add,
                replica_groups=[list(range(NCORES))],
                ins=[ar_in.opt()], outs=[ar_out.opt()])
            arres = cp.tile([P, 4 * B], F32, tag="arres")
            nc.sync.dma_start(out=arres[:], in_=ar_out[:])

            zt = cp.tile([P, 5 * B], F32, tag="zt")
            nc.vector.memset(zt[:], 0.0)
            for bi, (pre, ci, m) in enumerate(blocks):
                rec = grec_t if pre == "g" else srec_t
                nc.vector.tensor_tensor(
                    out=zt[:m, bi * B:(bi + 1) * B],
                    in0=arres[:m, bi * B:(bi + 1) * B],
                    in1=rec[:m, :], op=ALU.mult)
            nc.vector.tensor_copy(out=zt[:NF, 4 * B:5 * B], in_=pointT_t[:])

            # MLP feature-major: h1T [600,8] in 5 chunks, h2T [256,8] in 2
            h1 = cp.tile([P, 5 * B], F32, tag="h1")
            nc.vector.memset(h1[:], 0.0)
            for mchunk in range(5):
                m0, m1 = mchunk * P, min(600, (mchunk + 1) * P)
                hps = psB.tile([m1 - m0, B], F32, space="PSUM", tag="hps")
                for k in range(5):
                    nc.tensor.matmul(
                        out=hps[:],
                        lhsT=l1w_t[:, k * 600 + m0:k * 600 + m1],
                        rhs=zt[:, k * B:(k + 1) * B],
                        start=(k == 0), stop=(k == 4))
                nc.scalar.activation(h1[:m1 - m0, mchunk * B:(mchunk + 1) * B],
                                     hps[:], AF.Relu,
                                     bias=l1b_t[:m1 - m0, mchunk:mchunk + 1])
            h2 = cp.tile([P, 2 * B], F32, tag="h2")
            nc.vector.memset(h2[:], 0.0)
            for mchunk in range(2):
                m0, m1 = mchunk * P, (mchunk + 1) * P
                hps = psB.tile([P, B], F32, space="PSUM", tag="hps2")
                for k in range(5):
                    nc.tensor.matmul(
                        out=hps[:],
                        lhsT=l2w_t[:, k * 256 + m0:k * 256 + m1],
                        rhs=h1[:, k * B:(k + 1) * B],
                        start=(k == 0), stop=(k == 4))
                nc.scalar.activation(h2[:, mchunk * B:(mchunk + 1) * B],
                                     hps[:], AF.Relu,
                                     bias=l2b_t[:, mchunk:mchunk + 1])
            ops = psB.tile([NF, B], F32, space="PSUM", tag="ops")
            for k in range(2):
                nc.tensor.matmul(
                    out=ops[:], lhsT=l3w_t[:, k * NF:(k + 1) * NF],
                    rhs=h2[:, k * B:(k + 1) * B],
                    start=(k == 0), stop=(k == 1))
            o3sb = cp.tile([NF, B], F32, tag="o3sb")
            nc.scalar.activation(o3sb[:], ops[:], AF.Identity,
                                 bias=l3b_t[:NF, 0:1])
            tps = psT.tile([B, NF], F32, space="PSUM", tag="tfin")
            nc.tensor.transpose(out=tps[:], in_=o3sb[:],
                                identity=ident_t[:NF, :NF])
            osb = cp.tile([B, NF], F32, tag="osb")
            nc.scalar.copy(out=osb[:], in_=tps[:])
            nc.sync.dma_start(out=out_ext[:], in_=osb[:])

    nc.compile()
    return nc


# ---------------------------------------------------------------------------
# entry point
# ---------------------------------------------------------------------------

def kernel(**inputs):
    cfg = CFG_FULL
    inputs = {k: np.asarray(v) for k, v in inputs.items()}
    meta, in_maps = host_prep(inputs, cfg)
    nc = build_program(meta)
    trace = bool(int(os.environ.get("KERNEL_TRACE", "0")))
    if trace:
        import types
        from trn_agent_boot.trn_boot import _ntff_profile_via_ctypes
        hook = _ntff_profile_via_ctypes('/opt/axon/libaxon_pjrt.so')
        mod = types.ModuleType('antenv.axon_hooks')
        mod.get_axon_ntff_profile_hook = lambda: hook
        sys.modules['antenv.axon_hooks'] = mod
    res = run_bass_kernel_spmd(nc, in_maps, list(range(NCORES)), trace=trace)
    if trace and res.exec_time_ns:
        print(f"HW exec time: {res.exec_time_ns} ns")
    return np.asarray(res.results[0]["out"], np.float32)


# revision 5
# speedup vs baseline: 9.0146x; 9.0146x over previous
"""Trainium2 Bass kernel for DoubleGraphConvNet (gnn_message_passing).

Strategy (8 NeuronCores, SPMD single program):
- Nodes of each branch are dst-sharded across 8 cores using a padded
  numbering (core c owns padded rows [c*S, (c+1)*S)); the host builds a
  permutation balancing per-tile edge counts.
- segment_sum: per 128-edge chunk, indirect-DMA gather of source rows
  (bf16) + DVE is_equal one-hot + PSUM-accumulated matmul, producing the
  aggregate feature-major [c, 128] directly.
- Layers 1-2 aggregate-first, layer 3 transform-first (y3 = x3 @ W3r.T
  computed per-shard, then AllGather + gather of y3 rows).
- Cross-core exchange: AllGather of bf16 activations after L1 and of y3;
  final AllReduce of pooled sums; the small MLP is replicated on-device.
"""
import os
import sys
import math

sys.path.insert(0, "/opt/trn_rl_repo")

import numpy as np
import ml_dtypes

import concourse.bass as bass
import concourse.mybir as mybir
import concourse.tile as tile
from concourse import bacc
from concourse.bass_utils import run_bass_kernel_spmd

P = 128
NCORES = 8
B = 8
BF16 = mybir.dt.bfloat16
F32 = mybir.dt.float32
I32 = mybir.dt.int32
AF = mybir.ActivationFunctionType
ALU = mybir.AluOpType

CFG_FULL = dict(N_G=50000, E_G=800000, N_S=10000, E_S=160000, NF=64)


# ---------------------------------------------------------------------------
# host-side preprocessing
# ---------------------------------------------------------------------------

def _assign_nodes(dst, n_nodes):
    """Balanced node -> (core, slot, pos) assignment via greedy bin fill."""
    import heapq
    deg = np.bincount(dst, minlength=n_nodes)
    slots = math.ceil(n_nodes / (NCORES * P))
    nbins = NCORES * slots
    order = np.argsort(-deg, kind="stable")
    heap = [(0, 0, b) for b in range(nbins)]
    heapq.heapify(heap)
    bin_of = np.empty(n_nodes, np.int64)
    bin_fill = np.zeros(nbins, np.int64)
    pos_of = np.empty(n_nodes, np.int64)
    for n in order:
        while True:
            load, cnt, b = heapq.heappop(heap)
            if bin_fill[b] < P:
                break
        bin_of[n] = b
        pos_of[n] = bin_fill[b]
        bin_fill[b] += 1
        if bin_fill[b] < P:
            heapq.heappush(heap, (load + int(deg[n]), cnt + 1, b))
    bin_load = np.zeros(nbins, np.int64)
    np.add.at(bin_load, bin_of[dst], 1)
    core_of_bin = np.arange(nbins) % NCORES
    slot_of_bin = np.empty(nbins, np.int64)
    for c in range(NCORES):
        bins_c = np.where(core_of_bin == c)[0]
        order_c = bins_c[np.argsort(-bin_load[bins_c], kind="stable")]
        slot_of_bin[order_c] = np.arange(slots)
    core_of = core_of_bin[bin_of]
    slot_of = slot_of_bin[bin_of]
    perm = core_of * (slots * P) + slot_of * P + pos_of
    return perm, core_of, slot_of, pos_of, slots


def _build_edges(src, dst, perm, core_of, slot_of, pos_of, slots):
    ecore = core_of[dst]
    eslot = slot_of[dst]
    cnt = np.zeros((NCORES, slots), np.int64)
    np.add.at(cnt, (ecore, eslot), 1)
    nch = np.maximum(1, np.ceil(cnt.max(axis=0) / P).astype(np.int64))
    tc = int(nch.sum())
    starts = np.concatenate([[0], np.cumsum(nch)])[:-1]
    src_arr = np.zeros((NCORES, P, tc), np.int32)
    dstl_arr = np.full((NCORES, P, tc), 255.0, np.float32)
    order = np.lexsort((eslot, ecore))
    es, ed, ec, esl = src[order], dst[order], ecore[order], eslot[order]
    psrc = perm[es].astype(np.int32)
    pdst = pos_of[ed].astype(np.float32)
    key = ec * slots + esl
    bounds = np.searchsorted(key, np.arange(NCORES * slots + 1))
    for c in range(NCORES):
        for s in range(slots):
            k = c * slots + s
            a, b = bounds[k], bounds[k + 1]
            n = b - a
            if n == 0:
                continue
            c0 = int(starts[s])
            ncol = int(nch[s])
            buf_s = np.zeros(P * ncol, np.int32)
            buf_d = np.full(P * ncol, 255.0, np.float32)
            buf_s[:n] = psrc[a:b]
            buf_d[:n] = pdst[a:b]
            src_arr[c, :, c0:c0 + ncol] = buf_s.reshape(ncol, P).T
            dstl_arr[c, :, c0:c0 + ncol] = buf_d.reshape(ncol, P).T
    return src_arr, dstl_arr, nch.astype(int), tc


def _prep_branch(x, edge_index, batch, n_nodes):
    src = np.asarray(edge_index[0], np.int64)
    dst = np.asarray(edge_index[1], np.int64)
    perm, core_of, slot_of, pos_of, slots = _assign_nodes(dst, n_nodes)
    src_arr, dstl_arr, nch, tc = _build_edges(
        src, dst, perm, core_of, slot_of, pos_of, slots)
    npad = NCORES * slots * P
    S = slots * P
    nf = x.shape[1]
    x_full = np.zeros((npad, nf), np.float32)
    x_full[perm] = np.asarray(x, np.float32)
    xT = np.stack([x_full[c * S:(c + 1) * S].T.copy() for c in range(NCORES)])
    ohB = np.zeros((NCORES, P, slots * B), np.float32)
    bvec = np.asarray(batch, np.int64)
    pid = perm
    c_all, r_all = np.divmod(pid, S)
    s_all, p_all = np.divmod(r_all, P)
    for n in range(n_nodes):
        ohB[c_all[n], p_all[n], s_all[n] * B + int(bvec[n])] = 1.0
    cntb = np.bincount(bvec, minlength=B).astype(np.float32)
    recip = 1.0 / np.maximum(cntb, 1.0)
    return dict(perm=perm, slots=slots, S=S, npad=npad, nch=nch, tc=tc,
                src_arr=src_arr, dstl_arr=dstl_arr, x_full=x_full, xT=xT,
                ohB=ohB, recip=recip)


def _pack_wt(w):
    return np.ascontiguousarray(np.asarray(w, np.float32).T)


def _pack_bias(bvec, nchunks):
    out = np.zeros((P, nchunks), np.float32)
    b = np.asarray(bvec, np.float32)
    for j in range(nchunks):
        seg = b[j * P:(j + 1) * P]
        out[:len(seg), j] = seg
    return out


def host_prep(inputs, cfg):
    g = _prep_branch(inputs["graph_x"], inputs["graph_edge_index"],
                     inputs["graph_batch"], cfg["N_G"])
    s = _prep_branch(inputs["subgraph_x"], inputs["subgraph_edge_index"],
                     inputs["subgraph_batch"], cfg["N_S"])
    NF = cfg["NF"]
    meta = dict(g=g, s=s, NF=NF)

    bf = ml_dtypes.bfloat16
    common = {}
    common["iota"] = np.broadcast_to(
        np.arange(P, dtype=np.float32), (P, P)).copy()
    common["ident"] = np.eye(P, dtype=np.float32)

    dims = [(2 * NF, NF), (4 * NF, 2 * NF), (3 * NF, 4 * NF)]
    meta["dims"] = dims
    for pre in ("g", "s"):
        for li, (o, c) in enumerate(dims, start=1):
            common[f"{pre}W{li}rT"] = _pack_wt(inputs[f"{pre}W{li}r"]).astype(bf)
            common[f"{pre}W{li}nT"] = _pack_wt(inputs[f"{pre}W{li}n"]).astype(bf)
            common[f"{pre}B{li}"] = _pack_bias(inputs[f"{pre}B{li}"],
                                               math.ceil(o / P))
    O3 = 3 * NF
    zmap = np.full(5 * P, -1, np.int64)
    zmap[0:P] = np.arange(0, P)
    zmap[P:P + (O3 - P)] = np.arange(P, O3)
    zmap[2 * P:3 * P] = O3 + np.arange(0, P)
    zmap[3 * P:3 * P + (O3 - P)] = O3 + np.arange(P, O3)
    zmap[4 * P:4 * P + NF] = 2 * O3 + np.arange(NF)
    l1W = np.asarray(inputs["l1W"], np.float32)
    l1WT = np.zeros((5 * P, 600), np.float32)
    valid = zmap >= 0
    l1WT[valid] = l1W[:, zmap[valid]].T
    H1, H2 = 600, 256
    M1 = math.ceil(H1 / P)
    l2W = np.asarray(inputs["l2W"], np.float32)
    l2WT = np.zeros((M1 * P, H2), np.float32)
    l2WT[:H1] = l2W.T
    l3W = np.asarray(inputs["l3W"], np.float32)
    l3WT = np.ascontiguousarray(l3W.T)

    def pack_k(wt, kchunks, width):
        out = np.zeros((P, kchunks * width), np.float32)
        for k in range(kchunks):
            seg = wt[k * P:(k + 1) * P]
            out[:seg.shape[0], k * width:k * width + width] = seg
        return out

    common["l1WT"] = pack_k(l1WT, 5, 600)
    common["l2WT"] = pack_k(l2WT, M1, H2)
    common["l3WT"] = pack_k(l3WT, 2, NF)
    common["l1b"] = _pack_bias(inputs["l1b"], M1)
    common["l2b"] = _pack_bias(inputs["l2b"], 2)
    common["l3b"] = _pack_bias(inputs["l3b"], 1)
    common["pointT"] = np.ascontiguousarray(
        np.asarray(inputs["point"], np.float32).T)
    common["xg1_full"] = g["x_full"].astype(bf)
    common["xs1_full"] = s["x_full"].astype(bf)

    in_maps = []
    for c in range(NCORES):
        m = dict(common)
        m["g_src"] = g["src_arr"][c]
        m["g_dstl"] = g["dstl_arr"][c]
        m["s_src"] = s["src_arr"][c]
        m["s_dstl"] = s["dstl_arr"][c]
        m["xg1T"] = g["xT"][c].astype(bf)
        m["xs1T"] = s["xT"][c].astype(bf)
        m["g_ohB"] = g["ohB"][c]
        m["s_ohB"] = s["ohB"][c]
        m["g_recip"] = np.broadcast_to(g["recip"], (P, B)).astype(
            np.float32).copy()
        m["s_recip"] = np.broadcast_to(s["recip"], (P, B)).astype(
            np.float32).copy()
        in_maps.append(m)
    return meta, in_maps


# ---------------------------------------------------------------------------
# device program
# ---------------------------------------------------------------------------

def _ap3(t_ap, mid_count):
    """[P, X] AP -> [P, mid_count, X] with stride-0 middle dim."""
    return bass.AP(t_ap.tensor, t_ap.offset,
                   [list(t_ap.ap[0]), [0, mid_count], list(t_ap.ap[1])])


def build_program(meta):
    NF = meta["NF"]
    dims = meta["dims"]
    g, s = meta["g"], meta["s"]
    O3 = 3 * NF

    nc = bacc.Bacc(None, target_bir_lowering=False, debug=False)

    def din(name, shape, dtype):
        return nc.dram_tensor(name, list(shape), dtype, kind="ExternalInput")

    xg1_full = din("xg1_full", g["x_full"].shape, BF16)
    xs1_full = din("xs1_full", s["x_full"].shape, BF16)
    xg1T = din("xg1T", [NF, g["S"]], BF16)
    xs1T = din("xs1T", [NF, s["S"]], BF16)
    g_src = din("g_src", [P, g["tc"]], I32)
    g_dstl = din("g_dstl", [P, g["tc"]], F32)
    s_src = din("s_src", [P, s["tc"]], I32)
    s_dstl = din("s_dstl", [P, s["tc"]], F32)
    iota_in = din("iota", [P, P], F32)
    ident_in = din("ident", [P, P], F32)
    wts = {}
    for pre in ("g", "s"):
        for li, (o, c) in enumerate(dims, start=1):
            wts[f"{pre}W{li}rT"] = din(f"{pre}W{li}rT", [c, o], BF16)
            wts[f"{pre}W{li}nT"] = din(f"{pre}W{li}nT", [c, o], BF16)
            wts[f"{pre}B{li}"] = din(f"{pre}B{li}", [P, math.ceil(o / P)], F32)
    l1WT = din("l1WT", [P, 5 * 600], F32)
    l2WT = din("l2WT", [P, 5 * 256], F32)
    l3WT = din("l3WT", [P, 2 * NF], F32)
    l1b = din("l1b", [P, 5], F32)
    l2b = din("l2b", [P, 2], F32)
    l3b = din("l3b", [P, 1], F32)
    pointT = din("pointT", [NF, B], F32)
    g_ohB = din("g_ohB", [P, g["slots"] * B], F32)
    s_ohB = din("s_ohB", [P, s["slots"] * B], F32)
    g_recip = din("g_recip", [P, B], F32)
    s_recip = din("s_recip", [P, B], F32)

    out_ext = nc.dram_tensor("out", [B, NF], F32, kind="ExternalOutput")

    with tile.TileContext(nc) as tc:
        with tc.tile_pool(name="const", bufs=1) as cp, \
             tc.tile_pool(name="gat", bufs=3) as gat_p, \
             tc.tile_pool(name="oh", bufs=3) as oh_p, \
             tc.tile_pool(name="evac", bufs=2) as ev_p, \
             tc.tile_pool(name="elu", bufs=2) as elu_p, \
             tc.tile_pool(name="stage", bufs=2) as st_p, \
             tc.tile_pool(name="psA", bufs=2, space="PSUM") as psA, \
             tc.tile_pool(name="psB", bufs=2, space="PSUM") as psB, \
             tc.tile_pool(name="psT", bufs=2, space="PSUM") as psT, \
             tc.tile_pool(name="psP", bufs=1, space="PSUM") as psP, \
             tc.tile_pool(name="psM", bufs=1, space="PSUM") as psM, \
             tc.tile_pool(name="dram", bufs=1, space="DRAM") as dram:

            def load_const(name, src_t, shape, dtype):
                t = cp.tile(list(shape), dtype, tag=name)
                nc.sync.dma_start(out=t[:], in_=src_t[:])
                return t

            iota_t = load_const("iota", iota_in, [P, P], F32)
            ident_t = load_const("ident", ident_in, [P, P], F32)
            w_t = {}
            for pre in ("g", "s"):
                for li, (o, c) in enumerate(dims, start=1):
                    for rn in ("r", "n"):
                        nm = f"{pre}W{li}{rn}T"
                        kch = math.ceil(c / P)
                        t = cp.tile([P, kch * o], BF16, tag=nm)
                        src_w = wts[nm]
                        if c < P:
                            nc.sync.dma_start(out=t[:c, :o], in_=src_w[:])
                        else:
                            nc.sync.dma_start(
                                out=t[:].rearrange("p (k o) -> p k o", k=kch),
                                in_=src_w[:].rearrange("(k p) o -> p k o",
                                                       p=P))
                        w_t[nm] = t
                    nm = f"{pre}B{li}"
                    w_t[nm] = load_const(nm, wts[nm],
                                         [P, math.ceil(o / P)], F32)
            l1w_t = load_const("l1WT", l1WT, [P, 5 * 600], F32)
            l2w_t = load_const("l2WT", l2WT, [P, 5 * 256], F32)
            l3w_t = load_const("l3WT", l3WT, [P, 2 * NF], F32)
            l1b_t = load_const("l1b", l1b, [P, 5], F32)
            l2b_t = load_const("l2b", l2b, [P, 2], F32)
            l3b_t = load_const("l3b", l3b, [P, 1], F32)
            pointT_t = load_const("pointT", pointT, [NF, B], F32)
            gohB_t = load_const("g_ohB", g_ohB, [P, g["slots"] * B], F32)
            sohB_t = load_const("s_ohB", s_ohB, [P, s["slots"] * B], F32)
            grec_t = load_const("g_recip", g_recip, [P, B], F32)
            srec_t = load_const("s_recip", s_recip, [P, B], F32)
            gsrc_t = load_const("g_src", g_src, [P, g["tc"]], I32)
            gdstl_t = load_const("g_dstl", g_dstl, [P, g["tc"]], F32)
            ssrc_t = load_const("s_src", s_src, [P, s["tc"]], I32)
            sdstl_t = load_const("s_dstl", s_dstl, [P, s["tc"]], F32)

            xT_store = {}
            for pre, br in (("g", g), ("s", s)):
                for ab in "AB":
                    xT_store[pre + ab] = cp.tile(
                        [P, 2 * br["slots"] * P], BF16,
                        tag=f"xT{pre}{ab}", name=f"xT{pre}{ab}")
            nc.sync.dma_start(out=xT_store["gA"][:NF, :g["S"]], in_=xg1T[:])
            nc.sync.dma_start(out=xT_store["sA"][:NF, :s["S"]], in_=xs1T[:])

            xg2_shard = dram.tile([g["S"], 2 * NF], BF16, tag="xg2_shard")
            xg2_full = dram.tile([g["npad"], 2 * NF], BF16, tag="xg2_full")
            xs2_shard = dram.tile([s["S"], 2 * NF], BF16, tag="xs2_shard")
            xs2_full = dram.tile([s["npad"], 2 * NF], BF16, tag="xs2_full")
            yg3_shard = dram.tile([g["S"], O3], BF16, tag="yg3_shard")
            yg3_full = dram.tile([g["npad"], O3], BF16, tag="yg3_full")
            ys3_shard = dram.tile([s["S"], O3], BF16, tag="ys3_shard")
            ys3_full = dram.tile([s["npad"], O3], BF16, tag="ys3_full")
            ar_in = dram.tile([P, 4 * B], F32, tag="ar_in")
            ar_out = dram.tile([P, 4 * B], F32, tag="ar_out")

            # single pooled-accumulator psum tile: col block bi per
            # (branch, o-chunk): g0, g1, s0, s1
            pool_t = psP.tile([P, 4 * B], F32, space="PSUM", tag="pool")
            pool_bi = {("g", 0): 0, ("g", 1): 1, ("s", 0): 2, ("s", 1): 3}

            def emit_layer(pre, br, li, x_full_t, src_t, dstl_t,
                           xin_store, xout_store, shard_t):
                o, c = dims[li - 1]
                cg = c if li < 3 else O3
                och = math.ceil(o / P)
                kch = math.ceil(c / P)
                nch = br["nch"]
                starts = np.concatenate([[0], np.cumsum(nch)]).astype(int)
                WrT = w_t.get(f"{pre}W{li}rT")
                WnT = w_t[f"{pre}W{li}nT"]
                bias = w_t[f"{pre}B{li}"]
                ohB_t = gohB_t if pre == "g" else sohB_t

                for slot in range(br["slots"]):
                    n_j = int(nch[slot])
                    c0 = int(starts[slot])
                    g_t = gat_p.tile([P, n_j * cg], BF16, tag="gat")
                    for j in range(n_j):
                        nc.gpsimd.indirect_dma_start(
                            out=g_t[:, j * cg:(j + 1) * cg],
                            out_offset=None,
                            in_=x_full_t[:],
                            in_offset=bass.IndirectOffsetOnAxis(
                                ap=src_t[:, c0 + j:c0 + j + 1], axis=0))
                    oh_t = oh_p.tile([P, n_j * P], BF16, tag="oh")
                    d_ap = dstl_t[:, c0:c0 + n_j].to_broadcast([P, n_j, P])
                    i_ap = _ap3(iota_t[:], n_j)
                    nc.vector.tensor_tensor(
                        out=oh_t[:].rearrange("p (k q) -> p k q", k=n_j),
                        in0=d_ap, in1=i_ap, op=ALU.is_equal)

                    out_t = psB.tile([P, och * P], F32, space="PSUM",
                                     tag="out")
                    if li < 3:
                        agg_t = psA.tile([P, kch * P], F32, space="PSUM",
                                         tag="agg")
                        for j in range(n_j):
                            for kc in range(kch):
                                m0, m1 = kc * P, min(c, (kc + 1) * P)
                                nc.tensor.matmul(
                                    out=agg_t[:m1 - m0, kc * P:kc * P + P],
                                    lhsT=g_t[:, j * cg + m0:j * cg + m1],
                                    rhs=oh_t[:, j * P:(j + 1) * P],
                                    start=(j == 0), stop=(j == n_j - 1))
                        aggsb = ev_p.tile([P, kch * P], BF16, tag="aggsb")
                        for kc in range(kch):
                            m0, m1 = kc * P, min(c, (kc + 1) * P)
                            nc.scalar.copy(
                                out=aggsb[:m1 - m0, kc * P:kc * P + P],
                                in_=agg_t[:m1 - m0, kc * P:kc * P + P])
                        for oc in range(och):
                            o0, o1 = oc * P, min(o, (oc + 1) * P)
                            first = True
                            for kc in range(kch):
                                k0, k1 = kc * P, min(c, (kc + 1) * P)
                                nc.tensor.matmul(
                                    out=out_t[:o1 - o0, oc * P:oc * P + P],
                                    lhsT=WrT[:k1 - k0,
                                             kc * o + o0:kc * o + o1],
                                    rhs=aggsb[:k1 - k0, kc * P:kc * P + P],
                                    start=first, stop=False)
                                first = False
                                nc.tensor.matmul(
                                    out=out_t[:o1 - o0, oc * P:oc * P + P],
                                    lhsT=WnT[:k1 - k0,
                                             kc * o + o0:kc * o + o1],
                                    rhs=xin_store[:k1 - k0,
                                                  kc * br["slots"] * P
                                                  + slot * P:
                                                  kc * br["slots"] * P
                                                  + slot * P + P],
                                    start=False,
                                    stop=(kc == kch - 1))
                    else:
                        for j in range(n_j):
                            for oc in range(och):
                                o0, o1 = oc * P, min(o, (oc + 1) * P)
                                nc.tensor.matmul(
                                    out=out_t[:o1 - o0, oc * P:oc * P + P],
                                    lhsT=g_t[:, j * cg + o0:j * cg + o1],
                                    rhs=oh_t[:, j * P:(j + 1) * P],
                                    start=(j == 0), stop=False)
                        for oc in range(och):
                            o0, o1 = oc * P, min(o, (oc + 1) * P)
                            for kc in range(kch):
                                k0, k1 = kc * P, min(c, (kc + 1) * P)
                                nc.tensor.matmul(
                                    out=out_t[:o1 - o0, oc * P:oc * P + P],
                                    lhsT=WnT[:k1 - k0,
                                             kc * o + o0:kc * o + o1],
                                    rhs=xin_store[:k1 - k0,
                                                  kc * br["slots"] * P
                                                  + slot * P:
                                                  kc * br["slots"] * P
                                                  + slot * P + P],
                                    start=False,
                                    stop=(kc == kch - 1))

                    for oc in range(och):
                        o0, o1 = oc * P, min(o, (oc + 1) * P)
                        m = o1 - o0
                        pv = out_t[:m, oc * P:oc * P + P]
                        b_ap = bias[:m, oc:oc + 1]
                        tmin = elu_p.tile([P, P], F32, tag="tmin")
                        nc.vector.tensor_scalar(
                            out=tmin[:m, :], in0=pv, scalar1=b_ap,
                            scalar2=0.0, op0=ALU.add, op1=ALU.min)
                        texp = elu_p.tile([P, P], F32, tag="texp")
                        nc.scalar.activation(texp[:m, :], tmin[:m, :], AF.Exp)
                        trelu = elu_p.tile([P, P], F32, tag="trelu")
                        nc.scalar.activation(trelu[:m, :], pv, AF.Relu,
                                             bias=b_ap)
                        tsum = elu_p.tile([P, P], F32, tag="tsum")
                        nc.vector.tensor_tensor(
                            out=tsum[:m, :], in0=trelu[:m, :],
                            in1=texp[:m, :], op=ALU.add)
                        telu = elu_p.tile([P, P], F32, tag="telu")
                        nc.vector.tensor_scalar(
                            out=telu[:m, :], in0=tsum[:m, :],
                            scalar1=-1.0, scalar2=None, op0=ALU.add)
                        if li < 3:
                            nc.scalar.copy(
                                out=xout_store[:m,
                                               oc * br["slots"] * P
                                               + slot * P:
                                               oc * br["slots"] * P
                                               + slot * P + P],
                                in_=telu[:m, :])
                            if shard_t is not None:
                                tps = psT.tile([P, P], F32, space="PSUM",
                                               tag="tps")
                                nc.tensor.transpose(
                                    out=tps[:, :m], in_=telu[:m, :],
                                    identity=ident_t[:m, :m])
                                stg = st_p.tile([P, P], BF16, tag="stg")
                                nc.scalar.copy(out=stg[:, :m],
                                               in_=tps[:, :m])
                                nc.sync.dma_start(
                                    out=shard_t[slot * P:(slot + 1) * P,
                                                o0:o1],
                                    in_=stg[:, :m])
                        else:
                            tps = psT.tile([P, P], F32, space="PSUM",
                                           tag="tps")
                            nc.tensor.transpose(
                                out=tps[:, :m], in_=telu[:m, :],
                                identity=ident_t[:m, :m])
                            x4nm = st_p.tile([P, P], F32, tag="x4nm")
                            nc.scalar.copy(out=x4nm[:, :m], in_=tps[:, :m])
                            bi = pool_bi[(pre, oc)]
                            nc.tensor.matmul(
                                out=pool_t[:m, bi * B:(bi + 1) * B],
                                lhsT=x4nm[:, :m],
                                rhs=ohB_t[:, slot * B:(slot + 1) * B],
                                start=(slot == 0),
                                stop=(slot == br["slots"] - 1))

            def emit_y3(pre, br, xin_store, shard_t):
                o, c = dims[2]
                WrT = w_t[f"{pre}W3rT"]
                kch = math.ceil(c / P)
                och = math.ceil(o / P)
                for slot in range(br["slots"]):
                    y_t = psB.tile([P, och * P], F32, space="PSUM", tag="out")
                    for oc in range(och):
                        o0, o1 = oc * P, min(o, (oc + 1) * P)
                        for kc in range(kch):
                            k0, k1 = kc * P, min(c, (kc + 1) * P)
                            nc.tensor.matmul(
                                out=y_t[:o1 - o0, oc * P:oc * P + P],
                                lhsT=WrT[:k1 - k0, kc * o + o0:kc * o + o1],
                                rhs=xin_store[:k1 - k0,
                                              kc * br["slots"] * P + slot * P:
                                              kc * br["slots"] * P
                                              + slot * P + P],
                                start=(kc == 0), stop=(kc == kch - 1))
                    for oc in range(och):
                        o0, o1 = oc * P, min(o, (oc + 1) * P)
                        m = o1 - o0
                        ysb = elu_p.tile([P, P], F32, tag="ysb")
                        nc.scalar.copy(out=ysb[:m, :],
                                       in_=y_t[:m, oc * P:oc * P + P])
                        tps = psT.tile([P, P], F32, space="PSUM", tag="tps")
                        nc.tensor.transpose(out=tps[:, :m], in_=ysb[:m, :],
                                            identity=ident_t[:m, :m])
                        stg = st_p.tile([P, P], BF16, tag="stg")
                        nc.scalar.copy(out=stg[:, :m], in_=tps[:, :m])
                        nc.sync.dma_start(
                            out=shard_t[slot * P:(slot + 1) * P, o0:o1],
                            in_=stg[:, :m])

            def ag(shard_t, full_t):
                nc.gpsimd.collective_compute(
                    "AllGather", ALU.bypass,
                    replica_groups=[list(range(NCORES))],
                    ins=[shard_t.opt()], outs=[full_t.opt()])

            emit_layer("g", g, 1, xg1_full, gsrc_t, gdstl_t,
                       xT_store["gA"], xT_store["gB"], xg2_shard)
            ag(xg2_shard, xg2_full)
            emit_layer("s", s, 1, xs1_full, ssrc_t, sdstl_t,
                       xT_store["sA"], xT_store["sB"], xs2_shard)
            ag(xs2_shard, xs2_full)
            emit_layer("g", g, 2, xg2_full, gsrc_t, gdstl_t,
                       xT_store["gB"], xT_store["gA"], None)
            emit_y3("g", g, xT_store["gA"], yg3_shard)
            ag(yg3_shard, yg3_full)
            emit_layer("s", s, 2, xs2_full, ssrc_t, sdstl_t,
                       xT_store["sB"], xT_store["sA"], None)
            emit_y3("s", s, xT_store["sA"], ys3_shard)
            ag(ys3_shard, ys3_full)
            emit_layer("g", g, 3, yg3_full, gsrc_t, gdstl_t,
                       xT_store["gA"], None, None)
            emit_layer("s", s, 3, ys3_full, ssrc_t, sdstl_t,
                       xT_store["sA"], None, None)

            # pooled sums -> AllReduce -> recip -> zT -> MLP
            blocks = [("g", 0, P), ("g", 1, O3 - P), ("s", 0, P),
                      ("s", 1, O3 - P)]
            arsb = cp.tile([P, 4 * B], F32, tag="arsb")
            nc.vector.memset(arsb[:], 0.0)
            for bi, (pre, ci, m) in enumerate(blocks):
                nc.vector.tensor_copy(
                    out=arsb[:m, bi * B:(bi + 1) * B],
                    in_=pool_t[:m, bi * B:(bi + 1) * B])
            nc.sync.dma_start(out=ar_in[:], in_=arsb[:])
            nc.gpsimd.collective_compute(
                "AllReduce", ALU.add,
                replica_groups=[list(range(NCORES))],
                ins=[ar_in.opt()], outs=[ar_out.opt()])
            arres = cp.tile([P, 4 * B], F32, tag="arres")
            nc.sync.dma_start(out=arres[:], in_=ar_out[:])

            zt = cp.tile([P, 5 * B], F32, tag="zt")
            nc.vector.memset(zt[:], 0.0)
            for bi, (pre, ci, m) in enumerate(blocks):
                rec = grec_t if pre == "g" else srec_t
                zslot = bi  # zT slots 0..3 = pooled blocks, 4 = point
                nc.vector.tensor_tensor(
                    out=zt[:m, zslot * B:(zslot + 1) * B],
                    in0=arres[:m, bi * B:(bi + 1) * B],
                    in1=rec[:m, :], op=ALU.mult)
            nc.vector.tensor_copy(out=zt[:NF, 4 * B:5 * B], in_=pointT_t[:])

            h1 = cp.tile([P, 5 * B], F32, tag="h1")
            nc.vector.memset(h1[:], 0.0)
            for mchunk in range(5):
                m0, m1 = mchunk * P, min(600, (mchunk + 1) * P)
                hps = psM.tile([P, B], F32, space="PSUM", tag="hps")
                for k in range(5):
                    nc.tensor.matmul(
                        out=hps[:m1 - m0, :],
                        lhsT=l1w_t[:, k * 600 + m0:k * 600 + m1],
                        rhs=zt[:, k * B:(k + 1) * B],
                        start=(k == 0), stop=(k == 4))
                nc.scalar.activation(
                    h1[:m1 - m0, mchunk * B:(mchunk + 1) * B],
                    hps[:m1 - m0, :], AF.Relu,
                    bias=l1b_t[:m1 - m0, mchunk:mchunk + 1])
            h2 = cp.tile([P, 2 * B], F32, tag="h2")
            nc.vector.memset(h2[:], 0.0)
            for mchunk in range(2):
                m0 = mchunk * P
                hps = psM.tile([P, B], F32, space="PSUM", tag="hps")
                for k in range(5):
                    nc.tensor.matmul(
                        out=hps[:],
                        lhsT=l2w_t[:, k * 256 + m0:k * 256 + m0 + P],
                        rhs=h1[:, k * B:(k + 1) * B],
                        start=(k == 0), stop=(k == 4))
                nc.scalar.activation(
                    h2[:, mchunk * B:(mchunk + 1) * B], hps[:], AF.Relu,
                    bias=l2b_t[:, mchunk:mchunk + 1])
            ops = psM.tile([P, B], F32, space="PSUM", tag="hps")
            for k in range(2):
                nc.tensor.matmul(
                    out=ops[:NF, :], lhsT=l3w_t[:, k * NF:(k + 1) * NF],
                    rhs=h2[:, k * B:(k + 1) * B],
                    start=(k == 0), stop=(k == 1))
            o3sb = cp.tile([NF, B], F32, tag="o3sb")
            nc.scalar.activation(o3sb[:], ops[:NF, :], AF.Identity,
                                 bias=l3b_t[:NF, 0:1])
            tfin = psT.tile([P, P], F32, space="PSUM", tag="tps")
            nc.tensor.transpose(out=tfin[:B, :NF], in_=o3sb[:],
                                identity=ident_t[:NF, :NF])
            osb = cp.tile([B, NF], F32, tag="osb")
            nc.scalar.copy(out=osb[:], in_=tfin[:B, :NF])
            nc.sync.dma_start(out=out_ext[:], in_=osb[:])

    nc.compile()
    return nc


# ---------------------------------------------------------------------------
# entry point
# ---------------------------------------------------------------------------

def kernel(**inputs):
    cfg = CFG_FULL
    inputs = {k: np.asarray(v) for k, v in inputs.items()}
    meta, in_maps = host_prep(inputs, cfg)
    nc = build_program(meta)
    trace = bool(int(os.environ.get("KERNEL_TRACE", "0")))
    if trace:
        import types
        from trn_agent_boot.trn_boot import _ntff_profile_via_ctypes
        hook = _ntff_profile_via_ctypes('/opt/axon/libaxon_pjrt.so')
        mod = types.ModuleType('antenv.axon_hooks')
        mod.get_axon_ntff_profile_hook = lambda: hook
        sys.modules['antenv.axon_hooks'] = mod
    res = run_bass_kernel_spmd(nc, in_maps, list(range(NCORES)), trace=trace)
    if trace and res.exec_time_ns:
        print(f"HW exec time: {res.exec_time_ns} ns")
    return np.asarray(res.results[0]["out"], np.float32)
